# revision 1
# baseline (speedup 1.0000x reference)
"""Trainium2 Bass kernel for nn_BiLSTMModel (2-layer BiLSTM, B=1024 T=256 D=5 H=64).

Sharding: pure data parallel over batch across 8 cores (128 samples/core).

Transformed cell (validated vs the jax reference on host): all gates via
sigmoid (tanh(x) = 2*sigmoid(2x)-1 with the 2x folded into g-gate weight rows),
state s = 2c, h-tilde = h/2 (2x folded into W_hh / W_ih_l1 / fc_w):
    z  = W_ih@x_t + W_hh@ht_{t-1} + b      fp16 matmuls -> fp32 psum
    sz = sigmoid(z)                        one ACT op over all 4 gate regions
    p  = (g^ - 0.5) * s_i                  DVE STT
    s  = s_f * s + 4p                      DVE TT + STT
    c^ = sigmoid(s)                        ACT
    ht = (c^ - 0.5) * s_o                  DVE STT -> h-slab

Layout: directions stacked on partitions (fwd rows 0:64, bwd 64:128)
everywhere; psum bank [128, 4*B] holds gate regions [i|f|g|o]; per-gate M=64
matmuls (bwd via tile_position). The fp16 h-slab `comb` [128, T*B] holds both
directions' layer-0 output and feeds the recurrent and layer-1 input matmuls.
"""
import os
import numpy as np

import concourse.bacc as bacc
import concourse.bass as bass
import concourse.mybir as mybir
import concourse.tile as tile
from concourse.bass_utils import run_bass_kernel_spmd

H = 64
B = 128          # per-core batch
NCORES = 8
FULL_T = 256

F16 = mybir.dt.float16
F32 = mybir.dt.float32
AF = mybir.ActivationFunctionType
ALU = mybir.AluOpType


# ---------------------------------------------------------------- host packing

def _eff_dir(w_ih, w_hh, b_ih, b_hh, in_scale):
    """Effective weights for the transformed cell (float64 math).
    Gate row order stays PyTorch [i, f, g, o]."""
    Wi = np.asarray(w_ih, np.float64).copy() * in_scale
    Wh = np.asarray(w_hh, np.float64).copy() * 2.0
    b = (np.asarray(b_ih, np.float64) + np.asarray(b_hh, np.float64)).copy()
    g = slice(2 * H, 3 * H)
    Wi[g] *= 2.0
    Wh[g] *= 2.0
    b[g] *= 2.0
    return Wi, Wh, b


def make_core_inputs(inputs, T):
    w = {}
    bias = {0: np.zeros((8, 128)), 1: np.zeros((8, 128))}
    for di, (d, suf) in enumerate((("f", ""), ("b", "r"))):
        for l, scale in ((0, 1.0), (1, 2.0)):
            Wi, Wh, b = _eff_dir(inputs[f"w_ih_l{l}{suf}"], inputs[f"w_hh_l{l}{suf}"],
                                 inputs[f"b_ih_l{l}{suf}"], inputs[f"b_hh_l{l}{suf}"],
                                 scale)
            if l == 0:
                w[f"whh0{d}"] = Wh.T.astype(np.float16)   # [64, 256]
                w[f"wx0{d}"] = Wi.T.astype(np.float16)    # [5, 256]
            else:
                w[f"whh1{d}"] = Wh.T.astype(np.float16)   # [64, 256]
                w[f"wi1{d}"] = Wi.T.astype(np.float16)    # [128, 256]
            for g in range(4):
                bias[l][di * 4 + g, di * 64:(di + 1) * 64] = b[g * 64:(g + 1) * 64]
    w["bias0"] = bias[0].astype(np.float16)
    w["bias1"] = bias[1].astype(np.float16)
    mask8 = np.zeros((8, 4 * B), np.float16)
    for k in range(8):
        mask8[k, (k % 4) * B:((k % 4) + 1) * B] = 1.0
    w["mask8"] = mask8
    w["fcw"] = (2.0 * np.asarray(inputs["fc_w"], np.float64)).T.astype(np.float16)
    w["fcb"] = np.full((B, 1), float(np.asarray(inputs["fc_b"]).reshape(-1)[0]),
                       np.float32)

    x = np.asarray(inputs["x"])

    def core_map(k):
        xc = x[k * B:(k + 1) * B, :T, :]          # [B, T, 5]
        xo = np.ascontiguousarray(xc.transpose(2, 1, 0)).astype(np.float16)
        return {"xo": xo, **w}

    return core_map


# ---------------------------------------------------------------- device build

def build_nc(T=FULL_T, num_devices=NCORES, repeat=1):
    nc = bacc.Bacc("TRN2", target_bir_lowering=False, debug=False,
                   num_devices=num_devices)
    xo_d = nc.dram_tensor("xo", [5, T, B], F16, kind="ExternalInput")
    dshapes = {"whh0f": [64, 256], "whh0b": [64, 256],
               "wx0f": [5, 256], "wx0b": [5, 256],
               "wi1f": [128, 256], "wi1b": [128, 256],
               "whh1f": [64, 256], "whh1b": [64, 256],
               "bias0": [8, 128], "bias1": [8, 128], "mask8": [8, 4 * B]}
    wd = {n: nc.dram_tensor(n, s, F16, kind="ExternalInput")
          for n, s in dshapes.items()}
    fcw_d = nc.dram_tensor("fcw", [128, 1], F16, kind="ExternalInput")
    fcb_d = nc.dram_tensor("fcb", [B, 1], F32, kind="ExternalInput")
    out_d = nc.dram_tensor("out", [B, 1], F32, kind="ExternalOutput")

    ROW64 = {"whh0b", "whh1b"}   # bwd lhsT weights live on partitions 64:128

    with tile.TileContext(nc) as tc:
        with (
            tc.tile_pool(name="const", bufs=1) as cp,
            tc.tile_pool(name="wk", bufs=3) as wk,
            tc.tile_pool(name="ps", bufs=3, space="PSUM") as pp,
            tc.tile_pool(name="psfc", bufs=1, space="PSUM") as pfc,
        ):
            comb = cp.tile([128, T * B], F16, tag="comb")   # l0 h-slab, f/b stacked
            xos = cp.tile([5, T * B], F16, tag="xos")
            nc.sync.dma_start(xos[:], xo_d[:])
            W = {}
            for n in dshapes:
                if n in ROW64:
                    full = cp.tile([128, dshapes[n][1]], F16, tag=n, name=n)
                    W[n] = full[64:128, :]
                else:
                    W[n] = cp.tile(dshapes[n], F16, tag=n, name=n)[:]
                nc.sync.dma_start(W[n], wd[n][:])
            fcw_s = cp.tile([128, 1], F16, tag="fcw_s")
            nc.sync.dma_start(fcw_s[:], fcw_d[:])
            fcb_s = cp.tile([B, 1], F32, tag="fcb_s")
            nc.sync.dma_start(fcb_s[:], fcb_d[:])

            s_st = [cp.tile([128, B], F32, tag=f"s{p}", name=f"s{p}")
                    for p in (0, 1)]
            h1 = [cp.tile([128, B], F16, tag=f"h1{p}", name=f"h1{p}")
                  for p in (0, 1)]
            fcin = cp.tile([128, B], F16, tag="fcin")

            def cell_tail(S, par, l0_t=None, h1out=None):
                """S [128, 4B] sigmoid output -> state update -> h-tilde."""
                si, sf = S[:, 0:B], S[:, B:2 * B]
                gh, so = S[:, 2 * B:3 * B], S[:, 3 * B:4 * B]
                pt = wk.tile([128, B], F16, tag="pt")
                nc.vector.scalar_tensor_tensor(pt[:], gh, 0.5, si,
                                               ALU.subtract, ALU.mult)
                r = wk.tile([128, B], F32, tag="r")
                nc.vector.tensor_tensor(r[:], sf, s_st[1 - par][:], ALU.mult)
                nc.vector.scalar_tensor_tensor(s_st[par][:], pt[:], 4.0, r[:],
                                               ALU.mult, ALU.add)
                ch = wk.tile([128, B], F16, tag="ch")
                nc.scalar.activation(ch[:], s_st[par][:], AF.Sigmoid)
                if l0_t is not None:
                    tf, tb = l0_t
                    hof = comb[0:64, tf * B:(tf + 1) * B]
                    hob = comb[64:128, tb * B:(tb + 1) * B]
                else:
                    hof = h1out[0:64, :]
                    hob = h1out[64:128, :]
                nc.vector.scalar_tensor_tensor(hof, ch[0:64, :], 0.5,
                                               S[0:64, 3 * B:4 * B],
                                               ALU.subtract, ALU.mult)
                nc.vector.scalar_tensor_tensor(hob, ch[64:128, :], 0.5,
                                               S[64:128, 3 * B:4 * B],
                                               ALU.subtract, ALU.mult)

            # ================= layer 0 =================
            for _rep in range(repeat):
              nc.vector.memset(s_st[1][:], 0.0)
              for j in range(T):
                  tf, tb = j, T - 1 - j
                  par = j % 2
                  ps = pp.tile([128, 4 * B], F32, tag="ps")
                  mms = [(ps[:], W["bias0"][:], W["mask8"][:], None)]
                  for g in range(4):
                      gc = slice(g * 64, (g + 1) * 64)
                      mms.append((ps[0:64, g * B:(g + 1) * B], W["wx0f"][:, gc],
                                  xos[:, tf * B:(tf + 1) * B], None))
                      mms.append((ps[64:128, g * B:(g + 1) * B], W["wx0b"][:, gc],
                                  xos[:, tb * B:(tb + 1) * B], (0, 64)))
                  if j > 0:
                      for g in range(4):
                          gc = slice(g * 64, (g + 1) * 64)
                          mms.append((ps[0:64, g * B:(g + 1) * B],
                                      W["whh0f"][:, gc],
                                      comb[0:64, (tf - 1) * B:tf * B], None))
                          mms.append((ps[64:128, g * B:(g + 1) * B],
                                      W["whh0b"][:, gc],
                                      comb[64:128, (tb + 1) * B:(tb + 2) * B],
                                      (64, 64)))
                  for i, (o, l, rr, tp) in enumerate(mms):
                      nc.tensor.matmul(o, l, rr, start=(i == 0),
                                       stop=(i == len(mms) - 1),
                                       tile_position=tp,
                                       skip_group_check=True)
                  S = wk.tile([128, 4 * B], F16, tag="S")
                  nc.scalar.activation(S[:], ps[:], AF.Sigmoid)
                  cell_tail(S, par, l0_t=(tf, tb))

              # ================= layer 1 =================
              nc.vector.memset(s_st[1][:], 0.0)
              nc.vector.memset(h1[1][:], 0.0)
              for j in range(T):
                  tf, tb = j, T - 1 - j
                  par = j % 2
                  ps = pp.tile([128, 4 * B], F32, tag="ps")
                  mms = [(ps[:], W["bias1"][:], W["mask8"][:], None)]
                  for g in range(4):
                      gc = slice(g * 64, (g + 1) * 64)
                      mms.append((ps[0:64, g * B:(g + 1) * B], W["wi1f"][:, gc],
                                  comb[:, tf * B:(tf + 1) * B], None))
                      mms.append((ps[64:128, g * B:(g + 1) * B], W["wi1b"][:, gc],
                                  comb[:, tb * B:(tb + 1) * B], (0, 64)))
                  for g in range(4):
                      gc = slice(g * 64, (g + 1) * 64)
                      mms.append((ps[0:64, g * B:(g + 1) * B], W["whh1f"][:, gc],
                                  h1[1 - par][0:64, :], None))
                      mms.append((ps[64:128, g * B:(g + 1) * B], W["whh1b"][:, gc],
                                  h1[1 - par][64:128, :], (64, 64)))
                  for i, (o, l, rr, tp) in enumerate(mms):
                      nc.tensor.matmul(o, l, rr, start=(i == 0),
                                       stop=(i == len(mms) - 1),
                                       tile_position=tp,
                                       skip_group_check=True)
                  S = wk.tile([128, 4 * B], F16, tag="S")
                  nc.scalar.activation(S[:], ps[:], AF.Sigmoid)
                  cell_tail(S, par, h1out=h1[par])
                  if j == 0:
                      nc.vector.tensor_copy(fcin[64:128, :], h1[0][64:128, :])
              nc.vector.tensor_copy(fcin[0:64, :], h1[(T - 1) % 2][0:64, :])

            # ================= fc =================
            psf = pfc.tile([128, 1], F32, tag="psf")
            nc.tensor.matmul(psf[:], fcin[:], fcw_s[:], start=True, stop=True)
            outs = wk.tile([B, 1], F32, tag="outs")
            nc.scalar.activation(outs[:], psf[:], AF.Identity, bias=fcb_s[:])
            nc.sync.dma_start(out_d[:], outs[:])

    nc.compile()
    return nc


# ---------------------------------------------------------------- entry points

_NC_CACHE = {}


def _get_nc(T=FULL_T):
    if T not in _NC_CACHE:
        _NC_CACHE[T] = build_nc(T)
    return _NC_CACHE[T]


def kernel(**inputs):
    x = np.asarray(inputs["x"])
    T = x.shape[1]
    nc = _get_nc(T)
    core_map = make_core_inputs(inputs, T)
    in_maps = [core_map(k) for k in range(NCORES)]
    res = run_bass_kernel_spmd(nc, in_maps, list(range(NCORES)),
                               trace=bool(os.environ.get("BASS_TRACE_KERNEL")))
    out = np.concatenate([np.asarray(res.results[k]["out"]) for k in range(NCORES)],
                         axis=0)
    kernel.last_results = res
    return out.astype(np.float32)

